# revision 17
# baseline (speedup 1.0000x reference)
"""Trainium2 Bass kernel for batched attention (B=8, Lq=Lk=2048, D=Dv=128).

Sharding: pure data parallel - batch element b runs on NeuronCore b.

Per-core algorithm (v4 - engine-balanced restructure):

  Algebraic restructure removes both per-tensor projections from the hot path:
    scores^T = xk @ (Wk Wq^T) @ xq^T          (one fused weight W2)
      qT2 = W2 @ xq^T                          [d, Lq]  (prep, 2048 cols)
      sT_j = matmul(lhsT=xkT_j, rhs=qT2)       [128k, 1024q] per tile
    out = attn @ (xv @ Wv) = (attn @ xv) @ Wv  (defer Wv past the AV matmul)
      u = sum_j xv_j^T @ aT_j                  [d, 1024q] PSUM accum
      o_chunk = u_chunk^T @ Wv                 [128q, dv] (natural layout ->
                                                no output transposes)

  Mask handling never touches the hot loop: masked k rows are zeroed in the
  xk/xv input casts (fused per-partition multiply), so masked scores are 0,
  exp gives exactly 1, and the softmax denominator is corrected by the
  constant K = #masked positions (computed once in prep):
      den_q = sum_k exp~ - K
  The exp therefore needs no bias vector and runs as back-to-back FD=1024
  ACTIVATEs over a 3-slot PSUM rotation (2 slots of elasticity, so the
  scores matmuls hide under the previous exps instead of chaining).

  Inputs stream in quarters/halves over both DMA queues; casts (DVE) and
  PE transposes for the second half ride the psB slot during the first
  loop iterations, so the loop starts as soon as kT/qT2 half 0 are ready.
"""

import sys

sys.path.insert(0, "/opt/trn_rl_repo")

import numpy as np

import concourse.bass as bass
import concourse.mybir as mybir
import concourse.tile as tile
from concourse import bacc
from concourse.bass_utils import run_bass_kernel_spmd
from concourse.masks import make_identity

P = 128
L = 2048
D = 128
T = L // P  # 16 k-tiles
HQ = 1024  # q-half size
F32 = mybir.dt.float32
I32 = mybir.dt.int32
BF16 = mybir.dt.bfloat16
SCALE = 1.0 / float(np.sqrt(128.0))
N_CORES = 8

ADD = mybir.AluOpType.add
MULT = mybir.AluOpType.mult
SUB = mybir.AluOpType.subtract
NEQ = mybir.AluOpType.not_equal
EXP = mybir.ActivationFunctionType.Exp


def build():
    nc = bacc.Bacc("TRN2", target_bir_lowering=False, debug=False)

    q_ext = nc.declare_dram_parameter("query", [L, D], F32, isOutput=False)
    k_ext = nc.declare_dram_parameter("key", [L, D], F32, isOutput=False)
    v_ext = nc.declare_dram_parameter("value", [L, D], F32, isOutput=False)
    wq_ext = nc.declare_dram_parameter("Wq", [D, D], F32, isOutput=False)
    wk_ext = nc.declare_dram_parameter("Wk", [D, D], F32, isOutput=False)
    wv_ext = nc.declare_dram_parameter("Wv", [D, D], F32, isOutput=False)
    m_ext = nc.declare_dram_parameter("mask", [1, L], I32, isOutput=False)
    out_ext = nc.declare_dram_parameter("out", [L, D], BF16, isOutput=True)

    with tile.TileContext(nc) as tc:
        with (
            tc.tile_pool(name="const", bufs=1) as const,
            tc.tile_pool(name="big", bufs=1) as big,
            tc.tile_pool(name="stage", bufs=1) as stage,
            tc.tile_pool(name="att", bufs=9) as att,
            # score rotation: 3 x [128,1024]f32 tiles (per-tile dep tracking)
            tc.tile_pool(name="psA", bufs=3, space="PSUM") as psA,
            # 2-bank slot: h1 input prep -> u(h) -> dps(h) -> o(h) -> ...
            tc.tile_pool(name="psB", bufs=1, space="PSUM") as psB,
        ):
            # ---- PE warm-up + exp-table preload while DMAs start ----
            warm = const.tile([P, P], BF16, tag="warm")
            nc.gpsimd.memset(warm[:], 0.125)

            wstage = psA.tile([P, HQ], F32, tag="sc", name="wstage")
            wqT_ps = wstage[:, 0:P]
            wkT_ps = wstage[:, P : 2 * P]
            w2T_ps = wstage[:, 2 * P : 3 * P]
            warmfill = psB.tile([P, 512], F32, tag="ub", name="warmfill")

            def fillers(n):
                for _ in range(n):
                    nc.tensor.matmul(
                        warmfill[:, 0:P], warm[:], warm[:],
                        start=True, stop=True,
                    )

            fillers(16)
            dummy_exp = const.tile([P, 1], F32, tag="dummy")
            nc.scalar.activation(dummy_exp[:], warm[:, :1], EXP)

            # ---- input DMAs ----
            # natural layout [p, t, d]: row k = p*16 + t
            xf = {}
            srcs = {}
            for name, ext in (("k", k_ext), ("q", q_ext), ("v", v_ext)):
                xf[name] = stage.tile(
                    [P, T, D], F32, tag=f"xf_{name}", name=f"xf_{name}"
                )
                srcs[name] = ext[:].rearrange("(p t) d -> p t d", p=P)
            wf = {}
            for name in ("Wq", "Wk", "Wv"):
                wf[name] = stage.tile(
                    [P, D], F32, tag=f"wf_{name}", name=f"wf_{name}"
                )
            mask_i = const.tile([P, T], I32, tag="maski")

            # identity + constants first (gpsimd engine work precedes its DMAs)
            ident_f = stage.tile([P, P], F32, tag="identf")
            make_identity(nc, ident_f[:])
            ones_col = const.tile([P, 1], BF16, tag="ones")
            nc.gpsimd.memset(ones_col[:], 1.0)
            # gpsimd queue: mask + weights + h1/v halves
            nc.gpsimd.dma_start(
                mask_i[:], m_ext[:].rearrange("o (p t) -> p (o t)", p=P)
            )
            nc.gpsimd.dma_start(wf["Wq"][:], wq_ext[:])
            nc.gpsimd.dma_start(wf["Wk"][:], wk_ext[:])
            nc.gpsimd.dma_start(xf["k"][:, 8:16, :], srcs["k"][:, 8:16, :])
            nc.gpsimd.dma_start(xf["q"][:, 8:16, :], srcs["q"][:, 8:16, :])
            nc.gpsimd.dma_start(xf["v"][:, 0:8, :], srcs["v"][:, 0:8, :])
            nc.gpsimd.dma_start(xf["v"][:, 8:16, :], srcs["v"][:, 8:16, :])
            nc.gpsimd.dma_start(wf["Wv"][:], wv_ext[:])
            # sync queue: ONLY the two loop-gating halves (fewest instrs)
            nc.sync.dma_start(xf["q"][:, 0:8, :], srcs["q"][:, 0:8, :])
            nc.sync.dma_start(xf["k"][:, 0:8, :], srcs["k"][:, 0:8, :])

            # ---- prep: masks, weights ----
            ident_bf = const.tile([P, P], BF16, tag="identbf")
            nc.vector.tensor_copy(out=ident_bf[:], in_=ident_f[:])
            # additive exp bias: 0 where attend, -1e4 where masked
            mask_bias = const.tile([P, T], F32, tag="maskb")
            nc.vector.tensor_scalar(
                mask_bias[:], mask_i[:], 10000.0, -10000.0, MULT, ADD
            )

            nc.tensor.transpose(wqT_ps, wf["Wq"][:], ident_f[:])
            nc.tensor.transpose(wkT_ps, wf["Wk"][:], ident_f[:])
            wqT_bf = const.tile([P, D], BF16, tag="wqT")
            wkT_bf = const.tile([P, D], BF16, tag="wkT")
            nc.vector.tensor_copy(out=wqT_bf[:], in_=wqT_ps)
            nc.vector.tensor_copy(out=wkT_bf[:], in_=wkT_ps)
            fillers(4)
            # W2T = (WqT)^T @ WkT = Wq @ Wk^T  (so lhsT=W2T gives W2 @ x)
            nc.tensor.matmul(w2T_ps, wqT_bf[:], wkT_bf[:], start=True, stop=True)
            w2T_bf = const.tile([P, D], BF16, tag="w2T")
            nc.vector.tensor_copy(out=w2T_bf[:], in_=w2T_ps)

            wv_bf = const.tile([P, D], BF16, tag="wv_bf")
            nc.vector.tensor_copy(out=wv_bf[:], in_=wf["Wv"][:])

            # ---- h0 input casts + PE transposes (psum staging in sbig) ----
            xb = {}
            for name in ("k", "q", "v"):
                xb[name] = big.tile(
                    [P, T, D], BF16, tag=f"xb_{name}", name=f"xb_{name}"
                )
            xqT = big.tile([P, L], BF16, tag="xqT")
            xkT = big.tile([P, L], BF16, tag="xkT")
            qT2 = big.tile([P, L], BF16, tag="qT2")

            def cast_tiles(name, t0, t1):
                nc.vector.tensor_copy(
                    out=xb[name][:, t0:t1, :].rearrange("p a b -> p (a b)"),
                    in_=xf[name][:, t0:t1, :].rearrange("p a b -> p (a b)"),
                )

            def transpose_block(name, t0, t1, dstT, tview):
                # tview: [128, (t1-t0)*128] bf16 psum staging; one copy out
                for c, j in enumerate(range(t0, t1)):
                    nc.tensor.matmul(
                        tview[:, c * P : (c + 1) * P],
                        xb[name][:, j, :],
                        ident_bf[:],
                        is_transpose=True,
                        start=True,
                        stop=True,
                    )
                nc.vector.tensor_copy(
                    out=dstT[:, t0 * P : t1 * P], in_=tview
                )

            # h0: q then k (each: cast, transpose into own psA tile).
            # high_priority: this chain gates the loop start - make the
            # scheduler order it ahead of h1/v work on every engine.
            with tc.high_priority():
                qstage = psA.tile([P, HQ], F32, tag="sc", name="qstage")
                cast_tiles("q", 0, 8)
                transpose_block(
                    "q", 0, 8, xqT, qstage[:, 0:512].bitcast(BF16)
                )
                kstage = psA.tile([P, HQ], F32, tag="sc", name="kstage")
                cast_tiles("k", 0, 8)
                transpose_block(
                    "k", 0, 8, xkT, kstage[:, 0:512].bitcast(BF16)
                )
                # qT2 half 0 (ACT copies out while otherwise idle)
                qh0P = psA.tile([P, HQ], F32, tag="sc", name="qh0P")
                for c in range(2):
                    nc.tensor.matmul(
                        qh0P[:, c * 512 : (c + 1) * 512],
                        w2T_bf[:],
                        xqT[:, c * 512 : (c + 1) * 512],
                        start=True,
                        stop=True,
                    )
                nc.scalar.copy(out=qT2[:, 0:HQ], in_=qh0P[:])
            cast_tiles("v", 0, 8)

            # ---- main loop ----
            S_h = [
                big.tile([P, HQ], BF16, tag=f"S{h}", name=f"S{h}")
                for h in range(2)
            ]
            out_all = big.tile([P, T, D], BF16, tag="out_all")
            out_dst = out_ext[:].rearrange("(p t) d -> p t d", p=P)
            xv_m = xb["v"]

            def emit_scores(h, jl, sc):
                for c in range(2):
                    nc.tensor.matmul(
                        sc[:, c * 512 : (c + 1) * 512],
                        xkT[:, jl * P : (jl + 1) * P],
                        qT2[:, h * HQ + c * 512 : h * HQ + (c + 1) * 512],
                        start=True,
                        stop=True,
                    )

            def emit_u(u_ps, jl, a_t):
                for c in range(2):
                    nc.tensor.matmul(
                        u_ps[:, c * 512 : (c + 1) * 512],
                        xv_m[:, jl, :],
                        a_t[:, c * 512 : (c + 1) * 512],
                        start=(jl == 0),
                        stop=(jl == T - 1),
                    )

            def emit_S(h, jl, a_t):
                if jl == 0:
                    nc.vector.tensor_copy(out=S_h[h][:], in_=a_t[:])
                else:
                    nc.vector.tensor_tensor(S_h[h][:], S_h[h][:], a_t[:], ADD)

            # h1 input prep blocks, threaded through psB during early loop
            def h1_block_k():
                cast_tiles("k", 8, 16)
                tps = psB.tile([P, 512], F32, tag="ub", name="kh1T")
                transpose_block("k", 8, 16, xkT, tps[:].bitcast(BF16))

            def h1_block_q():
                cast_tiles("q", 8, 16)
                tps = psB.tile([P, 512], F32, tag="ub", name="qh1T")
                transpose_block("q", 8, 16, xqT, tps[:].bitcast(BF16))

            def h1_block_proj():
                pps = psB.tile([P, HQ], F32, tag="ub", name="qh1P")
                for c in range(2):
                    nc.tensor.matmul(
                        pps[:, c * 512 : (c + 1) * 512],
                        w2T_bf[:],
                        xqT[:, HQ + c * 512 : HQ + (c + 1) * 512],
                        start=True,
                        stop=True,
                    )
                nc.vector.tensor_copy(out=qT2[:, HQ : 2 * HQ], in_=pps[:])
                cast_tiles("v", 8, 16)

            # split epilogue for half h
            epi_state = {}

            def epi_stageA(h, u_ps, on_act=False):  # evacuate u
                u_bf = big.tile([P, HQ], BF16, tag=f"u_bf{h}", name=f"u_bf{h}")
                if on_act:
                    nc.scalar.copy(out=u_bf[:], in_=u_ps[:])
                else:
                    nc.vector.tensor_copy(out=u_bf[:], in_=u_ps[:])
                epi_state.setdefault(h, {})["u_bf"] = u_bf

            def epi_stageB(h, pool=None):  # PE: softmax denominators from S
                pool = pool or psB
                tag = "ub" if pool is psB else "sc"
                dps = pool.tile([P, 8], F32, tag=tag, name=f"dps{h}")
                epi_state.setdefault(h, {})
                for c in range(8):
                    nc.tensor.matmul(
                        dps[:, c : c + 1],
                        S_h[h][:, c * P : (c + 1) * P],
                        ones_col[:],
                        start=True,
                        stop=True,
                    )
                epi_state[h]["dps"] = dps

            def epi_stageC(h, pool=None):  # DVE recip + PE output chunks
                epi_state[h]["opool"] = pool or psB
                dps = epi_state[h]["dps"]
                denT = const.tile([P, 8], F32, tag=f"denT{h}", name=f"denT{h}")
                nc.vector.tensor_copy(out=denT[:], in_=dps[:])
                rT = const.tile([P, 8], F32, tag=f"rT{h}", name=f"rT{h}")
                nc.vector.reciprocal(rT[:], denT[:])
                o_ps = epi_state[h]["opool"].tile(
                    [P, HQ], F32,
                    tag="ub" if epi_state[h]["opool"] is psB else "sc",
                    name=f"o{h}",
                )
                u_bf = epi_state[h]["u_bf"]
                for c in range(8):
                    nc.tensor.matmul(
                        o_ps[:, c * P : (c + 1) * P],
                        u_bf[:, c * P : (c + 1) * P],
                        wv_bf[:],
                        start=True,
                        stop=True,
                    )
                epi_state[h].update(o_ps=o_ps, rT=rT)

            def epi_scales(h):
                # single DVE op: out = o * r with r broadcast along dv
                o_ps, rT = epi_state[h]["o_ps"], epi_state[h]["rT"]
                nc.vector.tensor_tensor(
                    out_all[:, h * 8 : (h + 1) * 8, :],
                    o_ps[:].rearrange("p (c v) -> p c v", c=8),
                    rT[:].to_broadcast([P, 8, P]),
                    MULT,
                )

            def epi_dma(h, g, eng):
                eng.dma_start(
                    out_dst[:, h * 8 + 4 * g : h * 8 + 4 * (g + 1), :],
                    out_all[:, h * 8 + 4 * g : h * 8 + 4 * (g + 1), :],
                )

            u_ps = {}
            pend = []  # [(h, jl, a_tile)] u-matmul work lagged behind exp

            def pop_u(n):
                for _ in range(n):
                    if not pend:
                        return
                    ph, pj, pa = pend.pop(0)
                    if ph not in u_ps:
                        u_ps[ph] = psB.tile(
                            [P, HQ], F32, tag="ub", name=f"u{ph}"
                        )
                    emit_u(u_ps[ph], pj, pa)

            for j in range(2 * T):
                h, jl = j // T, j % T
                sc = psA.tile([P, HQ], F32, tag="sc", name=f"sc{j}")
                emit_scores(h, jl, sc)
                a_t = att.tile([P, HQ], BF16, tag="aT", name=f"a{j}")
                nc.scalar.activation(
                    a_t[:], sc[:], EXP,
                    bias=mask_bias[:, jl : jl + 1], scale=SCALE,
                )
                if j == 0:
                    h1_block_k()
                elif j == 1:
                    h1_block_q()
                elif j == 2:
                    h1_block_proj()
                elif j < T:
                    pop_u(1)
                elif j == T:
                    epi_stageA(0, u_ps[0])
                    epi_stageB(0)
                elif j == T + 1:
                    epi_stageC(0)
                elif j == T + 2:
                    epi_scales(0)
                    epi_dma(0, 0, nc.gpsimd)
                    epi_dma(0, 1, nc.gpsimd)
                else:
                    pop_u(2)
                emit_S(h, jl, a_t)
                pend.append((h, jl, a_t))
                if j == T - 1:
                    pop_u(len(pend))  # close u(h0) before its epilogue
            pop_u(len(pend))
            scfill = psA.tile([P, HQ], F32, tag="sc", name="scfill")

            def tail_fillers(n):
                for _ in range(n):
                    nc.tensor.matmul(
                        scfill[:, 0:P], warm[:], warm[:], start=True, stop=True
                    )

            epi_stageB(1, pool=psA)
            epi_stageA(1, u_ps[1], on_act=True)
            tail_fillers(4)
            epi_stageC(1, pool=psA)
            epi_scales(1)
            epi_dma(1, 0, nc.sync)
            epi_dma(1, 1, nc.sync)
            tail_fillers(4)

    nc.compile()
    return nc


_NC_CACHE = None


def _get_nc():
    global _NC_CACHE
    if _NC_CACHE is None:
        _NC_CACHE = build()
    return _NC_CACHE


def kernel(query, key, value, Wq, Wk, Wv, attention_mask):
    query = np.asarray(query, dtype=np.float32)
    key = np.asarray(key, dtype=np.float32)
    value = np.asarray(value, dtype=np.float32)
    Wq = np.asarray(Wq, dtype=np.float32)
    Wk = np.asarray(Wk, dtype=np.float32)
    Wv = np.asarray(Wv, dtype=np.float32)
    mask = np.asarray(attention_mask, dtype=np.int32).reshape(N_CORES, 1, L)

    nc = _get_nc()
    in_maps = [
        {
            "query": np.ascontiguousarray(query[b]),
            "key": np.ascontiguousarray(key[b]),
            "value": np.ascontiguousarray(value[b]),
            "Wq": Wq,
            "Wk": Wk,
            "Wv": Wv,
            "mask": np.ascontiguousarray(mask[b]),
        }
        for b in range(N_CORES)
    ]
    res = run_bass_kernel_spmd(nc, in_maps, core_ids=list(range(N_CORES)))
    out = np.stack(
        [np.asarray(res.results[b]["out"]) for b in range(N_CORES)], axis=0
    )
    return out.astype(np.float32)


if __name__ == "__main__":
    rng = np.random.default_rng(0)
    q = rng.standard_normal((N_CORES, L, D), dtype=np.float32)
    k = rng.standard_normal((N_CORES, L, D), dtype=np.float32)
    v = rng.standard_normal((N_CORES, L, D), dtype=np.float32)
    wq = rng.standard_normal((128, 128), dtype=np.float32) * 0.08
    wk = rng.standard_normal((128, 128), dtype=np.float32) * 0.08
    wv = rng.standard_normal((128, 128), dtype=np.float32) * 0.08
    m = np.ones((N_CORES, 1, L), dtype=np.int32)
    out = kernel(
        query=q, key=k, value=v, Wq=wq, Wk=wk, Wv=wv, attention_mask=m
    )
    print(out.shape, out.dtype)


# revision 18
# speedup vs baseline: 1.0132x; 1.0132x over previous
"""Trainium2 Bass kernel for batched attention (B=8, Lq=Lk=2048, D=Dv=128).

Sharding: pure data parallel - batch element b runs on NeuronCore b.

Per-core algorithm (v4 - engine-balanced restructure):

  Algebraic restructure removes both per-tensor projections from the hot path:
    scores^T = xk @ (Wk Wq^T) @ xq^T          (one fused weight W2)
      qT2 = W2 @ xq^T                          [d, Lq]  (prep, 2048 cols)
      sT_j = matmul(lhsT=xkT_j, rhs=qT2)       [128k, 1024q] per tile
    out = attn @ (xv @ Wv) = (attn @ xv) @ Wv  (defer Wv past the AV matmul)
      u = sum_j xv_j^T @ aT_j                  [d, 1024q] PSUM accum
      o_chunk = u_chunk^T @ Wv                 [128q, dv] (natural layout ->
                                                no output transposes)

  Mask handling never touches the hot loop: masked k rows are zeroed in the
  xk/xv input casts (fused per-partition multiply), so masked scores are 0,
  exp gives exactly 1, and the softmax denominator is corrected by the
  constant K = #masked positions (computed once in prep):
      den_q = sum_k exp~ - K
  The exp therefore needs no bias vector and runs as back-to-back FD=1024
  ACTIVATEs over a 3-slot PSUM rotation (2 slots of elasticity, so the
  scores matmuls hide under the previous exps instead of chaining).

  Inputs stream in quarters/halves over both DMA queues; casts (DVE) and
  PE transposes for the second half ride the psB slot during the first
  loop iterations, so the loop starts as soon as kT/qT2 half 0 are ready.
"""

import sys

sys.path.insert(0, "/opt/trn_rl_repo")

import numpy as np

import concourse.bass as bass
import concourse.mybir as mybir
import concourse.tile as tile
from concourse import bacc
from concourse.bass_utils import run_bass_kernel_spmd
from concourse.masks import make_identity

P = 128
L = 2048
D = 128
T = L // P  # 16 k-tiles
HQ = 1024  # q-half size
F32 = mybir.dt.float32
I32 = mybir.dt.int32
BF16 = mybir.dt.bfloat16
SCALE = 1.0 / float(np.sqrt(128.0))
N_CORES = 8

ADD = mybir.AluOpType.add
MULT = mybir.AluOpType.mult
SUB = mybir.AluOpType.subtract
NEQ = mybir.AluOpType.not_equal
EXP = mybir.ActivationFunctionType.Exp


def build():
    nc = bacc.Bacc("TRN2", target_bir_lowering=False, debug=False)

    q_ext = nc.declare_dram_parameter("query", [L, D], F32, isOutput=False)
    k_ext = nc.declare_dram_parameter("key", [L, D], F32, isOutput=False)
    v_ext = nc.declare_dram_parameter("value", [L, D], F32, isOutput=False)
    wq_ext = nc.declare_dram_parameter("Wq", [D, D], F32, isOutput=False)
    wk_ext = nc.declare_dram_parameter("Wk", [D, D], F32, isOutput=False)
    wv_ext = nc.declare_dram_parameter("Wv", [D, D], F32, isOutput=False)
    m_ext = nc.declare_dram_parameter("mask", [1, L], I32, isOutput=False)
    out_ext = nc.declare_dram_parameter("out", [L, D], BF16, isOutput=True)

    with tile.TileContext(nc) as tc:
        with (
            tc.tile_pool(name="const", bufs=1) as const,
            tc.tile_pool(name="big", bufs=1) as big,
            tc.tile_pool(name="stage", bufs=1) as stage,
            tc.tile_pool(name="att", bufs=9) as att,
            # score rotation: 3 x [128,1024]f32 tiles (per-tile dep tracking)
            tc.tile_pool(name="psA", bufs=3, space="PSUM") as psA,
            # 2-bank slot: h1 input prep -> u(h) -> dps(h) -> o(h) -> ...
            tc.tile_pool(name="psB", bufs=1, space="PSUM") as psB,
        ):
            # ---- PE warm-up + exp-table preload while DMAs start ----
            warm = const.tile([P, P], BF16, tag="warm")
            nc.gpsimd.memset(warm[:], 0.125)

            wstage = psA.tile([P, HQ], F32, tag="sc", name="wstage")
            wqT_ps = wstage[:, 0:P]
            wkT_ps = wstage[:, P : 2 * P]
            w2T_ps = wstage[:, 2 * P : 3 * P]
            warmfill = psB.tile([P, 512], F32, tag="ub", name="warmfill")

            def fillers(n):
                for _ in range(n):
                    nc.tensor.matmul(
                        warmfill[:, 0:P], warm[:], warm[:],
                        start=True, stop=True,
                    )

            fillers(12)
            dummy_exp = const.tile([P, 1], F32, tag="dummy")
            nc.scalar.activation(dummy_exp[:], warm[:, :1], EXP)

            # ---- input DMAs ----
            # natural layout [p, t, d]: row k = p*16 + t
            xf = {}
            srcs = {}
            for name, ext in (("k", k_ext), ("q", q_ext), ("v", v_ext)):
                xf[name] = stage.tile(
                    [P, T, D], F32, tag=f"xf_{name}", name=f"xf_{name}"
                )
                srcs[name] = ext[:].rearrange("(p t) d -> p t d", p=P)
            wf = {}
            for name in ("Wq", "Wk", "Wv"):
                wf[name] = stage.tile(
                    [P, D], F32, tag=f"wf_{name}", name=f"wf_{name}"
                )
            mask_i = const.tile([P, T], I32, tag="maski")

            # identity + constants first (gpsimd engine work precedes its DMAs)
            ident_f = stage.tile([P, P], F32, tag="identf")
            make_identity(nc, ident_f[:])
            ones_col = const.tile([P, 1], BF16, tag="ones")
            nc.gpsimd.memset(ones_col[:], 1.0)
            # gpsimd queue: mask + weights + h1/v halves
            nc.gpsimd.dma_start(
                mask_i[:], m_ext[:].rearrange("o (p t) -> p (o t)", p=P)
            )
            nc.gpsimd.dma_start(wf["Wq"][:], wq_ext[:])
            nc.gpsimd.dma_start(wf["Wk"][:], wk_ext[:])
            nc.gpsimd.dma_start(xf["k"][:, 8:16, :], srcs["k"][:, 8:16, :])
            nc.gpsimd.dma_start(xf["q"][:, 8:16, :], srcs["q"][:, 8:16, :])
            nc.gpsimd.dma_start(xf["v"][:, 0:8, :], srcs["v"][:, 0:8, :])
            nc.gpsimd.dma_start(xf["v"][:, 8:16, :], srcs["v"][:, 8:16, :])
            nc.gpsimd.dma_start(wf["Wv"][:], wv_ext[:])
            # sync queue: ONLY the two loop-gating halves (fewest instrs)
            nc.sync.dma_start(xf["q"][:, 0:8, :], srcs["q"][:, 0:8, :])
            nc.sync.dma_start(xf["k"][:, 0:8, :], srcs["k"][:, 0:8, :])

            # ---- prep: masks, weights ----
            ident_bf = const.tile([P, P], BF16, tag="identbf")
            nc.vector.tensor_copy(out=ident_bf[:], in_=ident_f[:])
            # additive exp bias: 0 where attend, -1e4 where masked
            mask_bias = const.tile([P, T], F32, tag="maskb")
            nc.vector.tensor_scalar(
                mask_bias[:], mask_i[:], 10000.0, -10000.0, MULT, ADD
            )

            nc.tensor.transpose(wqT_ps, wf["Wq"][:], ident_f[:])
            nc.tensor.transpose(wkT_ps, wf["Wk"][:], ident_f[:])
            wqT_bf = const.tile([P, D], BF16, tag="wqT")
            wkT_bf = const.tile([P, D], BF16, tag="wkT")
            nc.vector.tensor_copy(out=wqT_bf[:], in_=wqT_ps)
            nc.vector.tensor_copy(out=wkT_bf[:], in_=wkT_ps)
            fillers(4)
            # W2T = (WqT)^T @ WkT = Wq @ Wk^T  (so lhsT=W2T gives W2 @ x)
            nc.tensor.matmul(w2T_ps, wqT_bf[:], wkT_bf[:], start=True, stop=True)
            w2T_bf = const.tile([P, D], BF16, tag="w2T")
            nc.vector.tensor_copy(out=w2T_bf[:], in_=w2T_ps)

            wv_bf = const.tile([P, D], BF16, tag="wv_bf")
            nc.vector.tensor_copy(out=wv_bf[:], in_=wf["Wv"][:])

            # ---- h0 input casts + PE transposes (psum staging in sbig) ----
            xb = {}
            for name in ("k", "q", "v"):
                xb[name] = big.tile(
                    [P, T, D], BF16, tag=f"xb_{name}", name=f"xb_{name}"
                )
            xqT = big.tile([P, L], BF16, tag="xqT")
            xkT = big.tile([P, L], BF16, tag="xkT")
            qT2 = big.tile([P, L], BF16, tag="qT2")

            def cast_tiles(name, t0, t1):
                nc.vector.tensor_copy(
                    out=xb[name][:, t0:t1, :].rearrange("p a b -> p (a b)"),
                    in_=xf[name][:, t0:t1, :].rearrange("p a b -> p (a b)"),
                )

            def transpose_block(name, t0, t1, dstT, tview):
                # tview: [128, (t1-t0)*128] bf16 psum staging; one copy out
                for c, j in enumerate(range(t0, t1)):
                    nc.tensor.matmul(
                        tview[:, c * P : (c + 1) * P],
                        xb[name][:, j, :],
                        ident_bf[:],
                        is_transpose=True,
                        start=True,
                        stop=True,
                    )
                nc.vector.tensor_copy(
                    out=dstT[:, t0 * P : t1 * P], in_=tview
                )

            # h0: q then k (each: cast, transpose into own psA tile).
            # high_priority: this chain gates the loop start - make the
            # scheduler order it ahead of h1/v work on every engine.
            with tc.high_priority():
                qstage = psA.tile([P, HQ], F32, tag="sc", name="qstage")
                cast_tiles("q", 0, 8)
                transpose_block(
                    "q", 0, 8, xqT, qstage[:, 0:512].bitcast(BF16)
                )
                kstage = psA.tile([P, HQ], F32, tag="sc", name="kstage")
                cast_tiles("k", 0, 8)
                transpose_block(
                    "k", 0, 8, xkT, kstage[:, 0:512].bitcast(BF16)
                )
                # qT2 half 0 (ACT copies out while otherwise idle)
                qh0P = psA.tile([P, HQ], F32, tag="sc", name="qh0P")
                for c in range(2):
                    nc.tensor.matmul(
                        qh0P[:, c * 512 : (c + 1) * 512],
                        w2T_bf[:],
                        xqT[:, c * 512 : (c + 1) * 512],
                        start=True,
                        stop=True,
                    )
                nc.scalar.copy(out=qT2[:, 0:HQ], in_=qh0P[:])
            cast_tiles("v", 0, 8)

            # ---- main loop ----
            S_h = [
                big.tile([P, HQ], BF16, tag=f"S{h}", name=f"S{h}")
                for h in range(2)
            ]
            out_all = big.tile([P, T, D], BF16, tag="out_all")
            out_dst = out_ext[:].rearrange("(p t) d -> p t d", p=P)
            xv_m = xb["v"]

            def emit_scores(h, jl, sc):
                for c in range(2):
                    nc.tensor.matmul(
                        sc[:, c * 512 : (c + 1) * 512],
                        xkT[:, jl * P : (jl + 1) * P],
                        qT2[:, h * HQ + c * 512 : h * HQ + (c + 1) * 512],
                        start=True,
                        stop=True,
                    )

            def emit_u(u_ps, jl, a_t):
                for c in range(2):
                    nc.tensor.matmul(
                        u_ps[:, c * 512 : (c + 1) * 512],
                        xv_m[:, jl, :],
                        a_t[:, c * 512 : (c + 1) * 512],
                        start=(jl == 0),
                        stop=(jl == T - 1),
                    )

            def emit_S(h, jl, a_t):
                if jl == 0:
                    nc.vector.tensor_copy(out=S_h[h][:], in_=a_t[:])
                else:
                    nc.vector.tensor_tensor(S_h[h][:], S_h[h][:], a_t[:], ADD)

            # h1 input prep blocks, threaded through psB during early loop
            def h1_block_k():
                cast_tiles("k", 8, 16)
                tps = psB.tile([P, 512], F32, tag="ub", name="kh1T")
                transpose_block("k", 8, 16, xkT, tps[:].bitcast(BF16))

            def h1_block_q():
                cast_tiles("q", 8, 16)
                tps = psB.tile([P, 512], F32, tag="ub", name="qh1T")
                transpose_block("q", 8, 16, xqT, tps[:].bitcast(BF16))

            def h1_block_proj():
                pps = psB.tile([P, HQ], F32, tag="ub", name="qh1P")
                for c in range(2):
                    nc.tensor.matmul(
                        pps[:, c * 512 : (c + 1) * 512],
                        w2T_bf[:],
                        xqT[:, HQ + c * 512 : HQ + (c + 1) * 512],
                        start=True,
                        stop=True,
                    )
                nc.vector.tensor_copy(out=qT2[:, HQ : 2 * HQ], in_=pps[:])
                cast_tiles("v", 8, 16)

            # split epilogue for half h
            epi_state = {}

            def epi_stageA(h, u_ps, on_act=False):  # evacuate u
                u_bf = big.tile([P, HQ], BF16, tag=f"u_bf{h}", name=f"u_bf{h}")
                if on_act:
                    nc.scalar.copy(out=u_bf[:], in_=u_ps[:])
                else:
                    nc.vector.tensor_copy(out=u_bf[:], in_=u_ps[:])
                epi_state[h] = {"u_bf": u_bf}

            def epi_stageB(h):  # PE: softmax denominators from S
                dps = psB.tile([P, 8], F32, tag="ub", name=f"dps{h}")
                for c in range(8):
                    nc.tensor.matmul(
                        dps[:, c : c + 1],
                        S_h[h][:, c * P : (c + 1) * P],
                        ones_col[:],
                        start=True,
                        stop=True,
                    )
                epi_state[h]["dps"] = dps

            def epi_stageC(h):  # DVE recip + PE output chunks o = u^T Wv
                dps = epi_state[h]["dps"]
                denT = const.tile([P, 8], F32, tag=f"denT{h}", name=f"denT{h}")
                nc.vector.tensor_copy(out=denT[:], in_=dps[:])
                rT = const.tile([P, 8], F32, tag=f"rT{h}", name=f"rT{h}")
                nc.vector.reciprocal(rT[:], denT[:])
                o_ps = psB.tile([P, HQ], F32, tag="ub", name=f"o{h}")
                u_bf = epi_state[h]["u_bf"]
                for c in range(8):
                    nc.tensor.matmul(
                        o_ps[:, c * P : (c + 1) * P],
                        u_bf[:, c * P : (c + 1) * P],
                        wv_bf[:],
                        start=True,
                        stop=True,
                    )
                epi_state[h].update(o_ps=o_ps, rT=rT)

            def epi_scales(h):
                # single DVE op: out = o * r with r broadcast along dv
                o_ps, rT = epi_state[h]["o_ps"], epi_state[h]["rT"]
                nc.vector.tensor_tensor(
                    out_all[:, h * 8 : (h + 1) * 8, :],
                    o_ps[:].rearrange("p (c v) -> p c v", c=8),
                    rT[:].to_broadcast([P, 8, P]),
                    MULT,
                )

            def epi_dma(h, g, eng):
                eng.dma_start(
                    out_dst[:, h * 8 + 4 * g : h * 8 + 4 * (g + 1), :],
                    out_all[:, h * 8 + 4 * g : h * 8 + 4 * (g + 1), :],
                )

            u_ps = {}
            pend = []  # [(h, jl, a_tile)] u-matmul work lagged behind exp

            def pop_u(n):
                for _ in range(n):
                    if not pend:
                        return
                    ph, pj, pa = pend.pop(0)
                    if ph not in u_ps:
                        u_ps[ph] = psB.tile(
                            [P, HQ], F32, tag="ub", name=f"u{ph}"
                        )
                    emit_u(u_ps[ph], pj, pa)

            for j in range(2 * T):
                h, jl = j // T, j % T
                sc = psA.tile([P, HQ], F32, tag="sc", name=f"sc{j}")
                emit_scores(h, jl, sc)
                a_t = att.tile([P, HQ], BF16, tag="aT", name=f"a{j}")
                nc.scalar.activation(
                    a_t[:], sc[:], EXP,
                    bias=mask_bias[:, jl : jl + 1], scale=SCALE,
                )
                if j == 0:
                    h1_block_k()
                elif j == 1:
                    h1_block_q()
                elif j == 2:
                    h1_block_proj()
                elif j < T:
                    pop_u(1)
                elif j == T:
                    epi_stageA(0, u_ps[0])
                    epi_stageB(0)
                elif j == T + 1:
                    epi_stageC(0)
                elif j == T + 2:
                    epi_scales(0)
                    epi_dma(0, 0, nc.gpsimd)
                    epi_dma(0, 1, nc.gpsimd)
                else:
                    pop_u(2)
                emit_S(h, jl, a_t)
                pend.append((h, jl, a_t))
                if j == T - 1:
                    pop_u(len(pend))  # close u(h0) before its epilogue
            pop_u(len(pend))
            scfill = psA.tile([P, HQ], F32, tag="sc", name="scfill")

            def tail_fillers(n):
                for _ in range(n):
                    nc.tensor.matmul(
                        scfill[:, 0:P], warm[:], warm[:], start=True, stop=True
                    )

            epi_stageA(1, u_ps[1], on_act=True)
            epi_stageB(1)
            tail_fillers(4)
            epi_stageC(1)
            epi_scales(1)
            epi_dma(1, 0, nc.sync)
            epi_dma(1, 1, nc.sync)
            tail_fillers(4)

    nc.compile()
    return nc


_NC_CACHE = None


def _get_nc():
    global _NC_CACHE
    if _NC_CACHE is None:
        _NC_CACHE = build()
    return _NC_CACHE


def kernel(query, key, value, Wq, Wk, Wv, attention_mask):
    query = np.asarray(query, dtype=np.float32)
    key = np.asarray(key, dtype=np.float32)
    value = np.asarray(value, dtype=np.float32)
    Wq = np.asarray(Wq, dtype=np.float32)
    Wk = np.asarray(Wk, dtype=np.float32)
    Wv = np.asarray(Wv, dtype=np.float32)
    mask = np.asarray(attention_mask, dtype=np.int32).reshape(N_CORES, 1, L)

    nc = _get_nc()
    in_maps = [
        {
            "query": np.ascontiguousarray(query[b]),
            "key": np.ascontiguousarray(key[b]),
            "value": np.ascontiguousarray(value[b]),
            "Wq": Wq,
            "Wk": Wk,
            "Wv": Wv,
            "mask": np.ascontiguousarray(mask[b]),
        }
        for b in range(N_CORES)
    ]
    res = run_bass_kernel_spmd(nc, in_maps, core_ids=list(range(N_CORES)))
    out = np.stack(
        [np.asarray(res.results[b]["out"]) for b in range(N_CORES)], axis=0
    )
    return out.astype(np.float32)


if __name__ == "__main__":
    rng = np.random.default_rng(0)
    q = rng.standard_normal((N_CORES, L, D), dtype=np.float32)
    k = rng.standard_normal((N_CORES, L, D), dtype=np.float32)
    v = rng.standard_normal((N_CORES, L, D), dtype=np.float32)
    wq = rng.standard_normal((128, 128), dtype=np.float32) * 0.08
    wk = rng.standard_normal((128, 128), dtype=np.float32) * 0.08
    wv = rng.standard_normal((128, 128), dtype=np.float32) * 0.08
    m = np.ones((N_CORES, 1, L), dtype=np.int32)
    out = kernel(
        query=q, key=k, value=v, Wq=wq, Wk=wk, Wv=wv, attention_mask=m
    )
    print(out.shape, out.dtype)
